# revision 6
# baseline (speedup 1.0000x reference)
"""Trainium2 Bass kernel: single-head causal attention.

  out[b] = softmax(mask((q[b]Wq+bq)(k[b]Wk+bk)^T / sqrt(dk))) (v[b]Wv+bv)

Sharding: data-parallel over batch, one batch element per NeuronCore (B=8,
n_cores=8). No collectives. Host-side prep is limited to layout (transpose
to [d_model, seq] so the d_model contraction sits on SBUF partitions) and
parameter re-layout / algebraic folding:
  - 1/sqrt(dk) is folded into Wq.
  - bk drops out (adds a per-query constant to scores -> softmax-invariant).
  - bq folds into an extra Wk column (Wk @ bq') against a ones-row in qpT.
  - bv is added after normalization (softmax rows sum to 1).

Per-core dataflow (S=2048, D=1024, dk=64, P=128):
  - qT,kT,vT [D,S] f32 DMA'd with in-flight f32->bf16 cast (SWDGE).
  - qpT [65,S] rows 0..63 = (Wq')^T qT (PE, 8 d-tile accumulation in PSUM),
    row 64 = ones (memset). kpT [65,S] = (Wk_aug)^T kT.
  - vp[t] [128,65] = v-tile @ Wv in natural layout via lhsT = vT-chunk;
    column 64 memset to 1 so the second matmul also emits the softmax
    denominator.
  - scoresT[sk-tile t] = kpT_t^T @ qpT for sq >= 128t (causal), exp on ACT
    from PSUM into bf16 u-tiles; diagonal 128x128 masked by 0/1 tri-mul.
  - out[sq-tile j] = sum_t u_t^T @ vp_t in PSUM [128,65]; normalize with
    vector.reciprocal + tensor_scalar_mul, then += bv, DMA out.
"""

import sys
from contextlib import ExitStack

import numpy as np

sys.path.insert(0, "/opt/trn_rl_repo")

import ml_dtypes  # noqa: E402

import concourse.bass as bass  # noqa: E402
from concourse import bacc  # noqa: E402
import concourse.mybir as mybir  # noqa: E402
import concourse.tile as tile  # noqa: E402
from concourse.bass import ds, ts  # noqa: E402
from concourse.bass_utils import run_bass_kernel_spmd  # noqa: E402

S = 2048
D = 1024
DK = 64
P = 128
NDT = D // P  # 8 d-model tiles
NST = S // P  # 16 seq tiles
CHUNK = 512  # matmul moving-operand / PSUM-bank free size
NCH = S // CHUNK  # 4 column chunks
B = 8
NCORES = 8

F32 = mybir.dt.float32
BF16 = mybir.dt.bfloat16
BF = ml_dtypes.bfloat16


def build(variant: str = "causal") -> bacc.Bacc:
    """variant: 'causal' (skip sk>sq tiles, tri-mask diagonal),
    'full' (no masking), 'general' (arbitrary multiplicative mask)."""
    assert variant in ("causal", "full", "general")
    causal = variant == "causal"

    nc = bacc.Bacc()
    qT_d = nc.declare_dram_parameter("qT", [D, S], F32, isOutput=False)
    kT_d = nc.declare_dram_parameter("kT", [D, S], F32, isOutput=False)
    vT_d = nc.declare_dram_parameter("vT", [D, S], F32, isOutput=False)
    wq_d = nc.declare_dram_parameter("wq", [P, NDT * DK], BF16, isOutput=False)
    wk_d = nc.declare_dram_parameter("wk", [P, NDT * (DK + 1)], BF16, isOutput=False)
    wv_d = nc.declare_dram_parameter("wv", [P, NDT * DK], BF16, isOutput=False)
    bvb_d = nc.declare_dram_parameter("bvb", [P, DK], F32, isOutput=False)
    if causal:
        m01_d = nc.declare_dram_parameter("m01", [P, P], BF16, isOutput=False)
    if variant == "general":
        mT_d = nc.declare_dram_parameter("mT", [S, S], BF16, isOutput=False)
    out_d = nc.declare_dram_parameter("out", [S, DK], F32, isOutput=True)

    with ExitStack() as ctx:
        tc = ctx.enter_context(tile.TileContext(nc))
        const_pool = ctx.enter_context(tc.tile_pool(name="const", bufs=1))
        ld_pool = ctx.enter_context(tc.tile_pool(name="loads", bufs=3))
        pp_pool = ctx.enter_context(tc.tile_pool(name="projT", bufs=1))
        u_pool = ctx.enter_context(tc.tile_pool(name="u", bufs=1))
        vp_pool = ctx.enter_context(tc.tile_pool(name="vp", bufs=1))
        osb_pool = ctx.enter_context(tc.tile_pool(name="osb", bufs=1))
        scr_pool = ctx.enter_context(tc.tile_pool(name="scr", bufs=1))
        ps_proj = ctx.enter_context(tc.tile_pool(name="ps_proj", bufs=2, space="PSUM"))
        ps_vp = ctx.enter_context(tc.tile_pool(name="ps_vp", bufs=2, space="PSUM"))
        ps_sc = ctx.enter_context(tc.tile_pool(name="ps_sc", bufs=2, space="PSUM"))
        ps_out = ctx.enter_context(tc.tile_pool(name="ps_out", bufs=2, space="PSUM"))

        # --- constants (HWDGE; separate DMA ring from the big SWDGE loads) ---
        wq_sb = const_pool.tile([P, NDT * DK], BF16, name="wq_sb")
        nc.sync.dma_start(wq_sb[:, :], wq_d[:, :])
        wk_sb = const_pool.tile([P, NDT * (DK + 1)], BF16, name="wk_sb")
        nc.sync.dma_start(wk_sb[:, :], wk_d[:, :])
        wv_sb = const_pool.tile([P, NDT * DK], BF16, name="wv_sb")
        nc.sync.dma_start(wv_sb[:, :], wv_d[:, :])
        bvb_sb = const_pool.tile([P, DK], F32, name="bvb_sb")
        nc.sync.dma_start(bvb_sb[:, :], bvb_d[:, :])
        if causal:
            m01_sb = const_pool.tile([P, P], BF16, name="m01_sb")
            nc.sync.dma_start(m01_sb[:, :], m01_d[:, :])

        # Early DVE "observation" reads of the consts, so steady-state DVE
        # ops downstream carry at most one sync-wait (walrus per-opcode
        # sync-wait slots are scarce, e.g. tensor_scalar has one).
        scr = scr_pool.tile([P, 4], F32, name="scr")
        nc.vector.tensor_copy(scr[:, ds(0, 1)], bvb_sb[:, ds(0, 1)])
        if causal:
            nc.vector.tensor_copy(scr[:, ds(1, 1)], m01_sb[:, ds(0, 1)])

        # --- big input loads, f32 -> bf16 cast in the DMA (SWDGE) ----------
        # SWDGE uses a single queue context, so these drain strictly in
        # program order: all of qT, then kT, then the vT column-chunks.
        qt = ld_pool.tile([P, NDT * S], BF16, tag="big", name="qt")
        kt = ld_pool.tile([P, NDT * S], BF16, tag="big", name="kt")
        vt = ld_pool.tile([P, NDT * S], BF16, tag="big", name="vt")
        qt3 = qt[:, :].rearrange("p (t s) -> p t s", s=S)
        kt3 = kt[:, :].rearrange("p (t s) -> p t s", s=S)
        vt3 = vt[:, :].rearrange("p (t s) -> p t s", s=S)
        nc.gpsimd.dma_start(qt3, qT_d[:, :].rearrange("(t p) s -> p t s", p=P))
        nc.gpsimd.dma_start(kt3, kT_d[:, :].rearrange("(t p) s -> p t s", p=P))
        vTr = vT_d[:, :].rearrange("(t p) s -> p t s", p=P)
        for c in range(NCH):
            nc.gpsimd.dma_start(
                vt3[:, :, ds(c * CHUNK, CHUNK)], vTr[:, :, ds(c * CHUNK, CHUNK)]
            )

        # --- q/k projections into [65, S] bf16 ------------------------------
        qpT = pp_pool.tile([DK + 1, S], BF16, tag="qpT", name="qpT")
        kpT = pp_pool.tile([DK + 1, S], BF16, tag="kpT", name="kpT")
        for src3, wsb, dst, m in ((qt3, wq_sb, qpT, DK), (kt3, wk_sb, kpT, DK + 1)):
            for c in range(NCH):
                ps = ps_proj.tile([DK + 1, CHUNK], F32, tag="ps_proj", name="ps_p")
                for d in range(NDT):
                    nc.tensor.matmul(
                        ps[:m, :],
                        lhsT=wsb[:, ts(d, m)],
                        rhs=src3[:, d, ds(c * CHUNK, CHUNK)],
                        start=(d == 0),
                        stop=(d == NDT - 1),
                    )
                nc.vector.tensor_copy(dst[:m, ds(c * CHUNK, CHUNK)], ps[:m, :])
        nc.vector.memset(qpT[ds(DK, 1), :], 1.0)

        # --- v projection straight to natural layout vp[t] [128, 65] -------
        vp_tiles = []
        for t in range(NST):
            ps = ps_vp.tile([P, DK], F32, tag="ps_vp", name="ps_v")
            for d in range(NDT):
                nc.tensor.matmul(
                    ps[:, :],
                    lhsT=vt3[:, d, ds(t * P, P)],
                    rhs=wv_sb[:, ts(d, DK)],
                    start=(d == 0),
                    stop=(d == NDT - 1),
                )
            vpt = vp_pool.tile([P, DK + 1], BF16, tag=f"vp{t}", name=f"vp{t}")
            nc.vector.tensor_copy(vpt[:, ds(0, DK)], ps[:, :])
            nc.vector.memset(vpt[:, ds(DK, 1)], 1.0)
            vp_tiles.append(vpt)

        # --- attention: per sk-tile scores+exp, then per sq-tile output ----
        if variant == "general":
            mT_tiles = []
            for t in range(NST):
                mt = u_pool.tile([P, S], BF16, tag=f"mT{t}", name=f"mT{t}")
                nc.sync.dma_start(mt[:, :], mT_d[ds(t * P, P), :])
                mT_tiles.append(mt)

        u_tiles = []
        for t in range(NST):
            lo = t * P if causal else 0  # first valid sq for this sk-tile
            ncols = S - lo
            ut = u_pool.tile([P, ncols], BF16, tag=f"ut{t}", name=f"ut{t}")
            # pieces of [lo, S) split on the CHUNK grid
            edges = [lo] + [e for e in range(0, S + 1, CHUNK) if e > lo]
            for a, b_ in zip(edges[:-1], edges[1:]):
                w = b_ - a
                ps = ps_sc.tile([P, CHUNK], F32, tag="ps_sc", name="ps_s")
                nc.tensor.matmul(
                    ps[:, :w],
                    lhsT=kpT[:, ds(t * P, P)],
                    rhs=qpT[:, ds(a, w)],
                    start=True,
                    stop=True,
                )
                nc.scalar.activation(
                    ut[:, ds(a - lo, w)],
                    ps[:, :w],
                    mybir.ActivationFunctionType.Exp,
                )
            if causal:
                # mask the diagonal 128x128 block: valid iff sk <= sq
                nc.vector.tensor_mul(ut[:, ds(0, P)], ut[:, ds(0, P)], m01_sb[:, :])
            elif variant == "general":
                nc.vector.tensor_mul(ut[:, :], ut[:, :], mT_tiles[t][:, :])
            u_tiles.append(ut)

            if causal:
                _emit_out_tile(
                    nc, osb_pool, ps_out, u_tiles, vp_tiles, bvb_sb, out_d, t, t + 1,
                    causal,
                )

        if not causal:
            for j in range(NST):
                _emit_out_tile(
                    nc, osb_pool, ps_out, u_tiles, vp_tiles, bvb_sb, out_d, j, NST,
                    causal,
                )

    nc.compile()
    return nc


def _emit_out_tile(nc, osb_pool, ps_out, u_tiles, vp_tiles, bvb_sb, out_d, j, lim, causal):
    """out[sq-tile j] = normalize(sum_{t<lim} u_t^T @ vp_t) + bv."""
    ops = ps_out.tile([P, DK + 1], F32, tag="ps_out", name="ps_o")
    for tt in range(lim):
        lo = tt * P if causal else 0
        nc.tensor.matmul(
            ops[:, :],
            lhsT=u_tiles[tt][:, ds(j * P - lo, P)],
            rhs=vp_tiles[tt][:, :],
            start=(tt == 0),
            stop=(tt == lim - 1),
        )
    rc = osb_pool.tile([P, 1], F32, tag=f"rc{j}", name=f"rc{j}")
    nc.vector.reciprocal(rc[:, :], ops[:, ds(DK, 1)])
    osb = osb_pool.tile([P, DK], F32, tag=f"osb{j}", name=f"osb{j}")
    nc.vector.tensor_scalar_mul(osb[:, :], ops[:, ds(0, DK)], rc[:, :])
    nc.vector.tensor_add(osb[:, :], osb[:, :], bvb_sb[:, :])
    nc.sync.dma_start(out_d[ds(j * P, P), :], osb[:, :])


def _host_prep(Wq, bq, Wk, bk, Wv, bv):
    scale = np.float32(1.0 / np.sqrt(np.float32(DK)))
    Wq = np.asarray(Wq, np.float32)
    Wk = np.asarray(Wk, np.float32)
    Wv = np.asarray(Wv, np.float32)
    bq = np.asarray(bq, np.float32)
    bv = np.asarray(bv, np.float32)

    def relay(w, m):
        return (
            w.reshape(NDT, P, m).transpose(1, 0, 2).reshape(P, NDT * m).astype(BF)
        )

    wq_r = relay(Wq * scale, DK)
    # bk is softmax-invariant (constant per query row) and dropped; bq folds
    # into an extra Wk column against the ones-row of qpT.
    wk_aug = np.concatenate([Wk, (Wk @ (bq * scale))[:, None]], axis=1)
    wk_r = relay(wk_aug, DK + 1)
    wv_r = relay(Wv, DK)
    bvb = np.ascontiguousarray(np.broadcast_to(bv, (P, DK)))
    return wq_r, wk_r, wv_r, bvb


_CACHE: dict = {}


def kernel(q, k, v, mask, Wq, bq, Wk, bk, Wv, bv):
    mask = np.asarray(mask)
    causal_ref = ~np.tril(np.ones((S, S), dtype=bool))
    if np.array_equal(mask, causal_ref):
        variant = "causal"
    elif not mask.any():
        variant = "full"
    else:
        variant = "general"

    wq_r, wk_r, wv_r, bvb = _host_prep(Wq, bq, Wk, bk, Wv, bv)
    m01 = np.triu(np.ones((P, P), np.float32)).astype(BF)

    in_maps = []
    for b in range(B):
        m = {
            "qT": np.ascontiguousarray(np.asarray(q[b], np.float32).T),
            "kT": np.ascontiguousarray(np.asarray(k[b], np.float32).T),
            "vT": np.ascontiguousarray(np.asarray(v[b], np.float32).T),
            "wq": wq_r,
            "wk": wk_r,
            "wv": wv_r,
            "bvb": bvb,
        }
        if variant == "causal":
            m["m01"] = m01
        if variant == "general":
            m["mT"] = np.ascontiguousarray((~mask).T.astype(BF))
        in_maps.append(m)

    if variant not in _CACHE:
        _CACHE[variant] = build(variant)
    nc = _CACHE[variant]

    res = run_bass_kernel_spmd(nc, in_maps, core_ids=list(range(NCORES)))
    out = np.stack([res.results[i]["out"] for i in range(NCORES)])
    return out.astype(np.float32)


# revision 8
# speedup vs baseline: 1.0512x; 1.0512x over previous
"""Trainium2 Bass kernel: single-head causal attention.

  out[b] = softmax(mask((q[b]Wq+bq)(k[b]Wk+bk)^T / sqrt(dk))) (v[b]Wv+bv)

Sharding: data-parallel over batch, one batch element per NeuronCore (B=8,
n_cores=8). No collectives. Host-side prep is limited to layout (transpose
to [d_model, seq] so the d_model contraction sits on SBUF partitions) and
parameter re-layout / algebraic folding:
  - 1/sqrt(dk) is folded into Wq.
  - bk drops out (adds a per-query constant to scores -> softmax-invariant).
  - bq folds into an extra Wk column (Wk @ bq') against a ones-row in qpT.
  - bv is added after normalization (softmax rows sum to 1).

Per-core dataflow (S=2048, D=1024, dk=64, P=128):
  - qT,kT,vT [D,S] f32 DMA'd with in-flight f32->bf16 cast (SWDGE).
  - qpT [65,S] rows 0..63 = (Wq')^T qT (PE, 8 d-tile accumulation in PSUM),
    row 64 = ones (memset). kpT [65,S] = (Wk_aug)^T kT.
  - vp[t] [128,65] = v-tile @ Wv in natural layout via lhsT = vT-chunk;
    column 64 memset to 1 so the second matmul also emits the softmax
    denominator.
  - scoresT[sk-tile t] = kpT_t^T @ qpT for sq >= 128t (causal), exp on ACT
    from PSUM into bf16 u-tiles; diagonal 128x128 masked by 0/1 tri-mul.
  - out[sq-tile j] = sum_t u_t^T @ vp_t in PSUM [128,65]; normalize with
    vector.reciprocal + tensor_scalar_mul, then += bv, DMA out.
"""

import sys
from contextlib import ExitStack

import numpy as np

sys.path.insert(0, "/opt/trn_rl_repo")

import ml_dtypes  # noqa: E402

import concourse.bass as bass  # noqa: E402
from concourse import bacc  # noqa: E402
import concourse.mybir as mybir  # noqa: E402
import concourse.tile as tile  # noqa: E402
from concourse.bass import ds, ts  # noqa: E402
from concourse.bass_utils import run_bass_kernel_spmd  # noqa: E402

S = 2048
D = 1024
DK = 64
P = 128
NDT = D // P  # 8 d-model tiles
NST = S // P  # 16 seq tiles
CHUNK = 512  # matmul moving-operand / PSUM-bank free size
NCH = S // CHUNK  # 4 column chunks
B = 8
NCORES = 8

F32 = mybir.dt.float32
BF16 = mybir.dt.bfloat16
BF = ml_dtypes.bfloat16


def build(variant: str = "causal") -> bacc.Bacc:
    """variant: 'causal' (skip sk>sq tiles, tri-mask diagonal),
    'full' (no masking), 'general' (arbitrary multiplicative mask)."""
    assert variant in ("causal", "full", "general")
    causal = variant == "causal"

    nc = bacc.Bacc()
    qT_d = nc.declare_dram_parameter("qT", [D, S], F32, isOutput=False)
    kT_d = nc.declare_dram_parameter("kT", [D, S], F32, isOutput=False)
    vT_d = nc.declare_dram_parameter("vT", [D, S], F32, isOutput=False)
    wq_d = nc.declare_dram_parameter("wq", [P, NDT * DK], BF16, isOutput=False)
    wk_d = nc.declare_dram_parameter("wk", [P, NDT * (DK + 1)], BF16, isOutput=False)
    wv_d = nc.declare_dram_parameter("wv", [P, NDT * DK], BF16, isOutput=False)
    bvb_d = nc.declare_dram_parameter("bvb", [P, DK], F32, isOutput=False)
    if causal:
        m01_d = nc.declare_dram_parameter("m01", [P, P], BF16, isOutput=False)
    if variant == "general":
        mT_d = nc.declare_dram_parameter("mT", [S, S], BF16, isOutput=False)
    out_d = nc.declare_dram_parameter("out", [S, DK], F32, isOutput=True)

    with ExitStack() as ctx:
        tc = ctx.enter_context(tile.TileContext(nc))
        const_pool = ctx.enter_context(tc.tile_pool(name="const", bufs=1))
        ld_pool = ctx.enter_context(tc.tile_pool(name="loads", bufs=3))
        pp_pool = ctx.enter_context(tc.tile_pool(name="projT", bufs=1))
        u_pool = ctx.enter_context(tc.tile_pool(name="u", bufs=1))
        vp_pool = ctx.enter_context(tc.tile_pool(name="vp", bufs=1))
        osb_pool = ctx.enter_context(tc.tile_pool(name="osb", bufs=1))
        scr_pool = ctx.enter_context(tc.tile_pool(name="scr", bufs=1))
        ps_proj = ctx.enter_context(tc.tile_pool(name="ps_proj", bufs=2, space="PSUM"))
        ps_vp = ctx.enter_context(tc.tile_pool(name="ps_vp", bufs=2, space="PSUM"))
        ps_sc = ctx.enter_context(tc.tile_pool(name="ps_sc", bufs=2, space="PSUM"))
        ps_out = ctx.enter_context(tc.tile_pool(name="ps_out", bufs=2, space="PSUM"))

        # --- constants (HWDGE; separate DMA ring from the big SWDGE loads) ---
        wq_sb = const_pool.tile([P, NDT * DK], BF16, name="wq_sb")
        nc.sync.dma_start(wq_sb[:, :], wq_d[:, :])
        wk_sb = const_pool.tile([P, NDT * (DK + 1)], BF16, name="wk_sb")
        nc.sync.dma_start(wk_sb[:, :], wk_d[:, :])
        wv_sb = const_pool.tile([P, NDT * DK], BF16, name="wv_sb")
        nc.sync.dma_start(wv_sb[:, :], wv_d[:, :])
        bvb_sb = const_pool.tile([P, DK], F32, name="bvb_sb")
        nc.sync.dma_start(bvb_sb[:, :], bvb_d[:, :])
        if causal:
            m01_sb = const_pool.tile([P, P], BF16, name="m01_sb")
            nc.sync.dma_start(m01_sb[:, :], m01_d[:, :])

        # Early DVE "observation" reads of the consts, so steady-state DVE
        # ops downstream carry at most one sync-wait (walrus per-opcode
        # sync-wait slots are scarce, e.g. tensor_scalar has one).
        scr = scr_pool.tile([P, 4], F32, name="scr")
        nc.vector.tensor_copy(scr[:, ds(0, 1)], bvb_sb[:, ds(0, 1)])
        if causal:
            nc.vector.tensor_copy(scr[:, ds(1, 1)], m01_sb[:, ds(0, 1)])

        # --- big input loads, f32 -> bf16 cast in the DMA (SWDGE) ----------
        # SWDGE uses a single queue context, so these drain strictly in
        # program order: kT, then qT, then vT, each in 4 column-chunks so
        # downstream compute starts per-chunk. k first (scores need all of
        # kpT), v last (only the output matmul needs it).
        qt = ld_pool.tile([P, NDT * S], BF16, tag="big", name="qt")
        kt = ld_pool.tile([P, NDT * S], BF16, tag="big", name="kt")
        vt = ld_pool.tile([P, NDT * S], BF16, tag="big", name="vt")
        qt3 = qt[:, :].rearrange("p (t s) -> p t s", s=S)
        kt3 = kt[:, :].rearrange("p (t s) -> p t s", s=S)
        vt3 = vt[:, :].rearrange("p (t s) -> p t s", s=S)
        for sb3, dr in ((kt3, kT_d), (qt3, qT_d), (vt3, vT_d)):
            dr3 = dr[:, :].rearrange("(t p) s -> p t s", p=P)
            for c in range(NCH):
                nc.gpsimd.dma_start(
                    sb3[:, :, ds(c * CHUNK, CHUNK)], dr3[:, :, ds(c * CHUNK, CHUNK)]
                )

        qpT = pp_pool.tile([DK + 1, S], BF16, tag="qpT", name="qpT")
        kpT = pp_pool.tile([DK + 1, S], BF16, tag="kpT", name="kpT")
        nc.vector.memset(qpT[ds(DK, 1), :], 1.0)

        def proj_chunk(src3, wsb, dst, m, c):
            ps = ps_proj.tile([DK + 1, CHUNK], F32, tag="ps_proj", name="ps_p")
            for d in range(NDT):
                nc.tensor.matmul(
                    ps[:m, :],
                    lhsT=wsb[:, ts(d, m)],
                    rhs=src3[:, d, ds(c * CHUNK, CHUNK)],
                    start=(d == 0),
                    stop=(d == NDT - 1),
                )
            nc.vector.tensor_copy(dst[:m, ds(c * CHUNK, CHUNK)], ps[:m, :])

        for c in range(NCH):
            proj_chunk(kt3, wk_sb, kpT, DK + 1, c)

        # --- scores + exp, sq-chunk-major so exp trails the q load ---------
        if variant == "general":
            mT_tiles = []
            for t in range(NST):
                mt = u_pool.tile([P, S], BF16, tag=f"mT{t}", name=f"mT{t}")
                nc.sync.dma_start(mt[:, :], mT_d[ds(t * P, P), :])
                mT_tiles.append(mt)

        u_tiles = []
        for t in range(NST):
            lo = t * P if causal else 0
            ut = u_pool.tile([P, S - lo], BF16, tag=f"ut{t}", name=f"ut{t}")
            u_tiles.append(ut)

        for c in range(NCH):
            proj_chunk(qt3, wq_sb, qpT, DK, c)
            t_hi = min(4 * c + 3, NST - 1) if causal else NST - 1
            for t in range(t_hi + 1):
                lo = t * P if causal else 0
                a = max(c * CHUNK, lo)
                w = (c + 1) * CHUNK - a
                ps = ps_sc.tile([P, CHUNK], F32, tag="ps_sc", name="ps_s")
                nc.tensor.matmul(
                    ps[:, :w],
                    lhsT=kpT[:, ds(t * P, P)],
                    rhs=qpT[:, ds(a, w)],
                    start=True,
                    stop=True,
                )
                ut = u_tiles[t]
                nc.scalar.activation(
                    ut[:, ds(a - lo, w)], ps[:, :w], mybir.ActivationFunctionType.Exp
                )
                if causal and a == lo:
                    # this piece starts at the diagonal 128x128 block
                    nc.vector.tensor_mul(ut[:, ds(0, P)], ut[:, ds(0, P)], m01_sb[:, :])
                elif variant == "general":
                    nc.vector.tensor_mul(
                        ut[:, ds(a, w)], ut[:, ds(a, w)], mT_tiles[t][:, ds(a, w)]
                    )

        # --- v projection + output tiles, per v column-chunk ---------------
        vp_tiles = []
        for c in range(NCH):
            for t in range(4 * c, 4 * c + 4):
                ps = ps_vp.tile([P, DK], F32, tag="ps_vp", name="ps_v")
                for d in range(NDT):
                    nc.tensor.matmul(
                        ps[:, :],
                        lhsT=vt3[:, d, ds(t * P, P)],
                        rhs=wv_sb[:, ts(d, DK)],
                        start=(d == 0),
                        stop=(d == NDT - 1),
                    )
                vpt = vp_pool.tile([P, DK + 1], BF16, tag=f"vp{t}", name=f"vp{t}")
                nc.vector.tensor_copy(vpt[:, ds(0, DK)], ps[:, :])
                nc.vector.memset(vpt[:, ds(DK, 1)], 1.0)
                vp_tiles.append(vpt)
            if causal:
                for j in range(4 * c, 4 * c + 4):
                    _emit_out_tile(
                        nc, osb_pool, ps_out, u_tiles, vp_tiles, bvb_sb, out_d,
                        j, j + 1, causal,
                    )
        if not causal:
            for j in range(NST):
                _emit_out_tile(
                    nc, osb_pool, ps_out, u_tiles, vp_tiles, bvb_sb, out_d,
                    j, NST, causal,
                )

    nc.compile()
    return nc


def _emit_out_tile(nc, osb_pool, ps_out, u_tiles, vp_tiles, bvb_sb, out_d, j, lim, causal):
    """out[sq-tile j] = normalize(sum_{t<lim} u_t^T @ vp_t) + bv."""
    ops = ps_out.tile([P, DK + 1], F32, tag="ps_out", name="ps_o")
    for tt in range(lim):
        lo = tt * P if causal else 0
        nc.tensor.matmul(
            ops[:, :],
            lhsT=u_tiles[tt][:, ds(j * P - lo, P)],
            rhs=vp_tiles[tt][:, :],
            start=(tt == 0),
            stop=(tt == lim - 1),
        )
    rc = osb_pool.tile([P, 1], F32, tag=f"rc{j}", name=f"rc{j}")
    nc.vector.reciprocal(rc[:, :], ops[:, ds(DK, 1)])
    osb = osb_pool.tile([P, DK], F32, tag=f"osb{j}", name=f"osb{j}")
    nc.vector.tensor_scalar_mul(osb[:, :], ops[:, ds(0, DK)], rc[:, :])
    nc.vector.tensor_add(osb[:, :], osb[:, :], bvb_sb[:, :])
    nc.sync.dma_start(out_d[ds(j * P, P), :], osb[:, :])


def _host_prep(Wq, bq, Wk, bk, Wv, bv):
    scale = np.float32(1.0 / np.sqrt(np.float32(DK)))
    Wq = np.asarray(Wq, np.float32)
    Wk = np.asarray(Wk, np.float32)
    Wv = np.asarray(Wv, np.float32)
    bq = np.asarray(bq, np.float32)
    bv = np.asarray(bv, np.float32)

    def relay(w, m):
        return (
            w.reshape(NDT, P, m).transpose(1, 0, 2).reshape(P, NDT * m).astype(BF)
        )

    wq_r = relay(Wq * scale, DK)
    # bk is softmax-invariant (constant per query row) and dropped; bq folds
    # into an extra Wk column against the ones-row of qpT.
    wk_aug = np.concatenate([Wk, (Wk @ (bq * scale))[:, None]], axis=1)
    wk_r = relay(wk_aug, DK + 1)
    wv_r = relay(Wv, DK)
    bvb = np.ascontiguousarray(np.broadcast_to(bv, (P, DK)))
    return wq_r, wk_r, wv_r, bvb


_CACHE: dict = {}


def kernel(q, k, v, mask, Wq, bq, Wk, bk, Wv, bv):
    mask = np.asarray(mask)
    causal_ref = ~np.tril(np.ones((S, S), dtype=bool))
    if np.array_equal(mask, causal_ref):
        variant = "causal"
    elif not mask.any():
        variant = "full"
    else:
        variant = "general"

    wq_r, wk_r, wv_r, bvb = _host_prep(Wq, bq, Wk, bk, Wv, bv)
    m01 = np.triu(np.ones((P, P), np.float32)).astype(BF)

    in_maps = []
    for b in range(B):
        m = {
            "qT": np.ascontiguousarray(np.asarray(q[b], np.float32).T),
            "kT": np.ascontiguousarray(np.asarray(k[b], np.float32).T),
            "vT": np.ascontiguousarray(np.asarray(v[b], np.float32).T),
            "wq": wq_r,
            "wk": wk_r,
            "wv": wv_r,
            "bvb": bvb,
        }
        if variant == "causal":
            m["m01"] = m01
        if variant == "general":
            m["mT"] = np.ascontiguousarray((~mask).T.astype(BF))
        in_maps.append(m)

    if variant not in _CACHE:
        _CACHE[variant] = build(variant)
    nc = _CACHE[variant]

    res = run_bass_kernel_spmd(nc, in_maps, core_ids=list(range(NCORES)))
    out = np.stack([res.results[i]["out"] for i in range(NCORES)])
    return out.astype(np.float32)
